# revision 13
# baseline (speedup 1.0000x reference)
"""CFConv (SchNet continuous-filter conv) Trainium2 Bass kernel, 8-core SPMD.

v2 strategy (filter-major MLP + SBUF-resident h):
  - Host: bucket edges by dest node range (ind_i // 6250 -> core), within a
    core group by (128-node dest window, src-half); group sizes padded to
    128 (window totals to 512) so the edge stream is supertiles of 512.
  - Device: h = x @ Win computed in bf16 and kept RESIDENT in SBUF
    ([128, 392, 128]: partition = node%128, rank = node//128). Neighbor
    rows are fetched with SBUF-source dma_gather (transpose mode), which
    lands them filter-major [128 filt, E] and avoids the HBM-latency-bound
    per-row gather entirely.
  - Filter MLP runs filter-major with resident stationary weights:
    z1 = Wf1.T@fT, e1 = Exp(z1+b1), a1 = Ln(e1+1) [= softplus],
    z2 = Wf2.T@a1, e2 = Exp(z2 + b2 - ln2*sum(Wf2,0) - ln2),
    tt = Ln(e2 + 0.5) [= ssp(z2+b2) - ln2, both ln2 shifts folded].
    Exp and Ln share one ACT table (patched table sets -> no table thrash).
  - m0 = tt * hg (filter-major), PE-transposed per 128-edge block to
    edge-major, then scattered into aggT[filt, dest] PSUM via one-hot
    matmuls (one-hot = is_equal(iota, l)*C built on DVE, cutoff folded).
  - Window output: out = Ln(Exp(aggT.T @ Wout - ln2) + 0.5) -> DMA.
No cross-core collectives: each core owns 6250 output rows.
"""

import math
import os
import sys

import numpy as np

sys.path.insert(0, "/opt/trn_rl_repo")

N_ATOMS = 50000
N_EDGES = 1600000
DIM = 128
NF = 128
NG = 50
CUTOFF = 10.0
LOG2 = float(np.log(2.0))
NCORES = 8
NPC = N_ATOMS // NCORES  # 6250 nodes per core
WIN = 128  # dest-window size
NWIN = (NPC + WIN - 1) // WIN  # 49
JHALF = 25088  # gather index half boundary (196*128, int16-safe)
SUPER = 512  # edges per supertile
NPAD = 50176  # padded atom count (392*128)
NRANK = NPAD // 128  # 392
LNCHUNK = 2048  # ACT Ln batch size


def _prep(inputs):
    """Host-side bucketing/padding. Returns per-core arrays + constants."""
    import ml_dtypes

    bf16 = np.dtype(ml_dtypes.bfloat16)

    x = np.asarray(inputs["x"], dtype=np.float32)
    r = np.asarray(inputs["r_ij"], dtype=np.float32)
    f = np.asarray(inputs["f_ij"], dtype=np.float32)
    ii = np.asarray(inputs["ind_i"]).astype(np.int64)
    jj = np.asarray(inputs["ind_j"]).astype(np.int64)

    core = ii // NPC
    wloc = (ii - core * NPC) // WIN  # 0..48
    lloc = (ii - core * NPC - wloc * WIN).astype(np.float32)  # 0..127
    half = (jj >= JHALF).astype(np.int64)
    NG_GROUPS = NWIN * 2
    wg = wloc * 2 + half

    gkey = core * NG_GROUPS + wg
    order = np.lexsort((jj, gkey))  # sort by (core, window, half), then j
    counts = np.bincount(gkey, minlength=NCORES * NG_GROUPS).reshape(
        NCORES, NG_GROUPS
    )
    gmax = counts.max(axis=0)
    gpad = np.maximum(WIN, ((gmax + WIN - 1) // WIN) * WIN)  # [98], 128-mult
    # force each window's (lo+hi) total to a SUPER multiple
    for w in range(NWIN):
        tot = gpad[2 * w] + gpad[2 * w + 1]
        gpad[2 * w + 1] += (-tot) % SUPER
    offs = np.concatenate([[0], np.cumsum(gpad)])
    E_pad = int(offs[-1])
    T_cols = E_pad // 128

    sorted_gkey = gkey[order]
    first_idx = np.searchsorted(sorted_gkey, np.arange(NCORES * NG_GROUPS))
    rank = np.arange(N_EDGES) - first_idx[sorted_gkey]
    slot = offs[sorted_gkey % NG_GROUPS] + rank

    per_core = []
    for c in range(NCORES):
        sel = order[core[order] == c]
        sl = slot[core[order] == c]
        f_pad = np.zeros((E_pad, NG), dtype=np.float32)
        r_pad = np.full(E_pad, 15.0, dtype=np.float32)  # killed by r<10 mask
        l_pad = np.zeros(E_pad, dtype=np.float32)
        j_pad = np.zeros(E_pad, dtype=np.int16)
        f_pad[sl] = f[sel]
        r_pad[sl] = r[sel]
        l_pad[sl] = lloc[sel]
        j_pad[sl] = (jj[sel] - half[sel] * JHALF).astype(np.int16)
        per_core.append(
            dict(
                fT=np.ascontiguousarray(f_pad.T.astype(bf16)),  # [50, E_pad]
                rA=np.ascontiguousarray(r_pad.reshape(T_cols, 128).T),
                lA=np.ascontiguousarray(l_pad.reshape(T_cols, 128).T),
                jx=np.ascontiguousarray(
                    np.tile(j_pad.reshape(-1, 16).T, (8, 1))
                ),  # [128, E_pad//16]
            )
        )

    xT = np.zeros((DIM, NPAD), dtype=np.float32)
    xT[:, :N_ATOMS] = x.T
    Wf2 = np.asarray(inputs["Wf2"], dtype=np.float32)
    b2e = (
        np.asarray(inputs["bf2"], dtype=np.float32)
        - LOG2 * Wf2.sum(axis=0)
        - LOG2
    ).reshape(NF, 1)
    bout = np.asarray(inputs["bout"], dtype=np.float32)
    consts = dict(
        xT=np.ascontiguousarray(xT.astype(bf16)),
        Wf1=np.asarray(inputs["Wf1"], dtype=np.float32).astype(bf16),
        Wf2=Wf2.astype(bf16),
        Win=np.ascontiguousarray(
            np.asarray(inputs["Win"], dtype=np.float32).astype(bf16)
        ),
        Wout=np.ascontiguousarray(
            np.asarray(inputs["Wout"], dtype=np.float32).astype(bf16)
        ),
        b1=np.asarray(inputs["bf1"], dtype=np.float32).reshape(NF, 1),
        b2e=np.ascontiguousarray(b2e),
        boutr=np.ascontiguousarray(bout.reshape(1, NF)),
        ones=np.ones((1, 128), dtype=np.float32),
        iota=np.ascontiguousarray(
            np.broadcast_to(np.arange(128, dtype=np.float32), (128, 128))
            .astype(bf16)
            .copy()
        ),
        ident=np.eye(128, dtype=np.float32).astype(bf16),
    )
    return per_core, consts, gpad, E_pad, T_cols


def _patch_act_tables():
    """Strip Exp/Ln/Sin from every act table set except the combined
    natural_log_exp_and_others (Exp+Ln) and trig_and_small (Sin), so the
    table chooser never alternates tables between Exp and Ln ops. Set ids
    are positional, so keys/order are preserved."""
    import concourse.bacc as bacc_mod
    import concourse.mybir as mybir

    if getattr(bacc_mod, "_cfconv_act_patched", False):
        return
    AF = mybir.ActivationFunctionType
    orig = bacc_mod.get_activation_tables

    def patched(arch):
        tabs = orig(arch)
        strip = {AF.Exp, AF.Ln, AF.Sin}
        out = {}
        for name, fns in tabs.items():
            if name in ("natural_log_exp_and_others", "trig_and_small"):
                out[name] = fns
            else:
                out[name] = fns - strip
        return out

    bacc_mod.get_activation_tables = patched
    bacc_mod._cfconv_act_patched = True


def _build(gpad, E_pad, T_cols, bout_nonzero=False, no_gather=False):
    """Build the SPMD bass program (same for all cores)."""
    from contextlib import ExitStack

    import concourse.bacc as bacc
    import concourse.bass as bass
    import concourse.mybir as mybir
    import concourse.tile as tile

    _patch_act_tables()

    dt = mybir.dt
    AF = mybir.ActivationFunctionType
    OP = mybir.AluOpType

    nc = bacc.Bacc()

    # ---- I/O ----
    fT_d = nc.declare_dram_parameter("fT", [NG, E_pad], dt.bfloat16, isOutput=False)
    rA_d = nc.declare_dram_parameter("rA", [128, T_cols], dt.float32, isOutput=False)
    lA_d = nc.declare_dram_parameter("lA", [128, T_cols], dt.float32, isOutput=False)
    jx_d = nc.declare_dram_parameter(
        "jx", [128, E_pad // 16], dt.int16, isOutput=False
    )
    xT_d = nc.declare_dram_parameter("xT", [DIM, NPAD], dt.bfloat16, isOutput=False)
    Wf1_d = nc.declare_dram_parameter("Wf1", [NG, NF], dt.bfloat16, isOutput=False)
    Wf2_d = nc.declare_dram_parameter("Wf2", [NF, NF], dt.bfloat16, isOutput=False)
    Win_d = nc.declare_dram_parameter("Win", [DIM, NF], dt.bfloat16, isOutput=False)
    Wout_d = nc.declare_dram_parameter("Wout", [NF, NF], dt.bfloat16, isOutput=False)
    b1_d = nc.declare_dram_parameter("b1", [NF, 1], dt.float32, isOutput=False)
    b2e_d = nc.declare_dram_parameter("b2e", [NF, 1], dt.float32, isOutput=False)
    bout_d = nc.declare_dram_parameter("boutr", [1, NF], dt.float32, isOutput=False)
    ones_d = nc.declare_dram_parameter("ones", [1, 128], dt.float32, isOutput=False)
    iota_d = nc.declare_dram_parameter("iota", [128, 128], dt.bfloat16, isOutput=False)
    ident_d = nc.declare_dram_parameter(
        "ident", [128, 128], dt.bfloat16, isOutput=False
    )
    out_d = nc.declare_dram_parameter("out", [NPC, NF], dt.float32, isOutput=True)

    offs = np.concatenate([[0], np.cumsum(gpad)]).astype(int)
    JRANK = JHALF // 128  # 196

    with tile.TileContext(nc) as tc, ExitStack() as ctx:
        cpool = ctx.enter_context(tc.tile_pool(name="consts", bufs=1))
        meta = ctx.enter_context(tc.tile_pool(name="meta", bufs=1))
        scratch = ctx.enter_context(tc.tile_pool(name="scratch", bufs=1))
        scratch2 = ctx.enter_context(tc.tile_pool(name="scratch2", bufs=1))
        xm = ctx.enter_context(tc.tile_pool(name="xm", bufs=3))
        hpool = ctx.enter_context(tc.tile_pool(name="hsb", bufs=1))
        hgpool = ctx.enter_context(tc.tile_pool(name="hg", bufs=2))
        ftpool = ctx.enter_context(tc.tile_pool(name="ft", bufs=3))
        e1pool = ctx.enter_context(tc.tile_pool(name="e1w", bufs=1))
        e2pool = ctx.enter_context(tc.tile_pool(name="e2w", bufs=1))
        mepool = ctx.enter_context(tc.tile_pool(name="m0e", bufs=2))
        ohpool = ctx.enter_context(tc.tile_pool(name="oh", bufs=6))
        opool = ctx.enter_context(tc.tile_pool(name="outs", bufs=3))
        pz = ctx.enter_context(
            tc.tile_pool(name="pz", bufs=2, space=bass.MemorySpace.PSUM)
        )
        pz2 = ctx.enter_context(
            tc.tile_pool(name="pz2", bufs=2, space=bass.MemorySpace.PSUM)
        )
        pmt = ctx.enter_context(
            tc.tile_pool(name="pmt", bufs=2, space=bass.MemorySpace.PSUM)
        )
        pagg = ctx.enter_context(
            tc.tile_pool(name="pagg", bufs=2, space=bass.MemorySpace.PSUM)
        )

        # ---- constants ----
        Wf1 = cpool.tile([NG, NF], dt.bfloat16)
        nc.sync.dma_start(Wf1[:], Wf1_d[:])
        Wf2 = cpool.tile([NF, NF], dt.bfloat16)
        nc.sync.dma_start(Wf2[:], Wf2_d[:])
        Win = cpool.tile([DIM, NF], dt.bfloat16)
        nc.sync.dma_start(Win[:], Win_d[:])
        Wout = cpool.tile([NF, NF], dt.bfloat16)
        nc.sync.dma_start(Wout[:], Wout_d[:])
        b1 = cpool.tile([NF, 1], dt.float32)
        nc.sync.dma_start(b1[:], b1_d[:])
        b2e = cpool.tile([NF, 1], dt.float32)
        nc.sync.dma_start(b2e[:], b2e_d[:])
        boutr = cpool.tile([1, NF], dt.float32)
        nc.sync.dma_start(boutr[:], bout_d[:])
        ones = cpool.tile([1, 128], dt.float32)
        nc.sync.dma_start(ones[:], ones_d[:])
        iota = cpool.tile([128, 128], dt.bfloat16)
        nc.sync.dma_start(iota[:], iota_d[:])
        ident = cpool.tile([128, 128], dt.bfloat16)
        nc.sync.dma_start(ident[:], ident_d[:])
        chalf = cpool.tile([128, 1], dt.float32)
        nc.gpsimd.memset(chalf[:], 0.5)
        cmln2 = cpool.tile([128, 1], dt.float32)
        nc.gpsimd.memset(cmln2[:], -LOG2)

        # ---- per-edge metadata: l, C ----
        lA = meta.tile([128, T_cols], dt.float32)
        nc.sync.dma_start(lA[:], lA_d[:])
        rA = scratch.tile([128, T_cols], dt.float32)
        nc.sync.dma_start(rA[:], rA_d[:])
        jx = meta.tile([128, E_pad // 16], dt.int16)
        nc.sync.dma_start(jx[:], jx_d[:])

        CA = meta.tile([128, T_cols], dt.float32)
        # cos(pi*r/10) = sin(pi/2 - pi*r/10), arg in [-pi, pi] for r in
        # [0, 15]; C = (0.5*C0+0.5) * (r < 10).  Sin first -> trig table
        # loads once, everything after uses the Exp+Ln table.
        # In-place: rA <- pi/2 - pi*r/10; then r<10 <=> rA > -pi/2.
        nc.vector.tensor_scalar(
            rA[:], rA[:], float(-np.pi / CUTOFF), float(np.pi / 2), OP.mult, OP.add
        )
        nc.scalar.activation(CA[:], rA[:], AF.Sin)
        nc.vector.tensor_scalar(CA[:], CA[:], 0.5, 0.5, OP.mult, OP.add)
        msk = scratch2.tile([128, T_cols], dt.bfloat16)
        nc.vector.tensor_scalar(
            msk[:], rA[:], float(-np.pi / 2), None, OP.is_gt
        )
        nc.vector.tensor_tensor(CA[:], CA[:], msk[:], OP.mult)

        # ---- phase 1: h = x @ Win, bf16, resident in SBUF ----
        # h_lo/h_hi[p, r, :] = h[(rbase + r)*128 + p, :] (token layout,
        # tpr=128). Two tiles: a single source view must stay under 64 KiB
        # of free bytes per partition, and view offsets into the source are
        # not honored by the HW gather path.
        h_lo = hpool.tile([128, JRANK, NF], dt.bfloat16, tag="hlo")
        h_hi = hpool.tile([128, NRANK - JRANK, NF], dt.bfloat16, tag="hhi")
        for nb4 in range(NRANK // 4):  # 98 groups of 4 node-blocks
            xa = xm.tile([128, 4, 128], dt.bfloat16)
            nc.sync.dma_start(
                xa[:],
                xT_d[:, nb4 * 512 : (nb4 + 1) * 512].rearrange(
                    "p (b n) -> p b n", b=4
                ),
            )
            hp = pz.tile([128, 4, NF], dt.float32, tag="z")
            for b in range(4):
                nc.tensor.matmul(
                    hp[:, b, :], xa[:, b, :], Win[:], start=True, stop=True
                )
            r0 = nb4 * 4
            if r0 + 4 <= JRANK:
                nc.vector.tensor_copy(h_lo[:, r0 : r0 + 4, :], hp[:])
            else:
                nc.vector.tensor_copy(
                    h_hi[:, r0 - JRANK : r0 - JRANK + 4, :], hp[:]
                )

        tc.strict_bb_all_engine_barrier()

        # ---- phase 2: edge loop, one window (128 dest nodes) at a time ----
        for w in range(NWIN):
            g0 = int(gpad[2 * w])
            g1 = int(gpad[2 * w + 1])
            woff = int(offs[2 * w])
            wsize = g0 + g1
            ns = wsize // SUPER

            hg = hgpool.tile([128, 1, wsize], dt.bfloat16)
            if no_gather:
                nc.gpsimd.memset(hg[:], 1.0)
            else:
                # chunk to <=512 idxs: single_packet=True packs the SWDGE
                # ring into multi-descriptor packets (the per-descriptor
                # ring-fetch turnaround otherwise caps throughput at
                # ~100 descs/us), but breaks above 512 idxs per call.
                for src_t, hoff, hsz in ((h_lo, 0, g0), (h_hi, g0, g1)):
                    for c0 in range(0, hsz, 512):
                        csz = min(512, hsz - c0)
                        e0 = woff + hoff + c0
                        nc.gpsimd.dma_gather(
                            hg[:, :, hoff + c0 : hoff + c0 + csz],
                            src_t[:],
                            jx[:, e0 // 16 : (e0 + csz) // 16],
                            csz,
                            csz,
                            NF,
                            transpose=True,
                            sbuf_tokens_per_rank=128,
                            sbuf_free_dim_per_rank=NF * 2,
                        )
            # stage 1: z1 = Wf1.T @ ft ; e1 = Exp(z1 + b1)
            e1w = e1pool.tile([128, wsize], dt.bfloat16)
            for s in range(ns):
                sl = slice(s * SUPER, (s + 1) * SUPER)
                ft = ftpool.tile([NG, SUPER], dt.bfloat16)
                nc.sync.dma_start(
                    ft[:], fT_d[:, woff + s * SUPER : woff + (s + 1) * SUPER]
                )
                z1 = pz.tile([128, SUPER], dt.float32, tag="z")
                nc.tensor.matmul(z1[:], Wf1[:], ft[:], start=True, stop=True)
                nc.scalar.activation(e1w[:, sl], z1[:], AF.Exp, bias=b1[:, 0:1])
            # stage 2: a1 = Ln(e1 + 1)  [in-place, batched]
            for c0 in range(0, wsize, LNCHUNK):
                c1 = min(c0 + LNCHUNK, wsize)
                nc.scalar.activation(
                    e1w[:, c0:c1], e1w[:, c0:c1], AF.Ln, bias=1.0
                )
            # stage 3: z2 = Wf2.T @ a1 ; e2 = Exp(z2 + b2e)
            e2w = e2pool.tile([128, wsize], dt.bfloat16)
            for s in range(ns):
                sl = slice(s * SUPER, (s + 1) * SUPER)
                z2 = pz2.tile([128, SUPER], dt.float32)
                nc.tensor.matmul(z2[:], Wf2[:], e1w[:, sl], start=True, stop=True)
                nc.scalar.activation(e2w[:, sl], z2[:], AF.Exp, bias=b2e[:, 0:1])
            # stage 4: tt = Ln(e2 + 0.5)  [= ssp(z2+b2) - ln2, in-place]
            for c0 in range(0, wsize, LNCHUNK):
                c1 = min(c0 + LNCHUNK, wsize)
                nc.scalar.activation(
                    e2w[:, c0:c1], e2w[:, c0:c1], AF.Ln, bias=chalf[:, 0:1]
                )

            # stage 5: m0 = tt * hg; transpose to edge-major; scatter
            aggT = pagg.tile([128, 128], dt.float32)
            n_tiles_w = wsize // 128
            for s in range(ns):
                sl = slice(s * SUPER, (s + 1) * SUPER)
                m0f = xm.tile([128, SUPER], dt.bfloat16)
                nc.vector.tensor_tensor(
                    m0f[:], e2w[:, sl], hg[:, 0, sl], OP.mult
                )
                m0T = pmt.tile([128, SUPER], dt.bfloat16)
                for b in range(4):
                    bs = slice(b * 128, (b + 1) * 128)
                    nc.tensor.transpose(m0T[:, bs], m0f[:, bs], ident[:])
                m0e = mepool.tile([128, SUPER], dt.bfloat16)
                nc.vector.tensor_copy(m0e[:], m0T[:])
                for b in range(4):
                    bs = slice(b * 128, (b + 1) * 128)
                    tcol = (woff + s * SUPER) // 128 + b
                    oh = ohpool.tile([128, 128], dt.bfloat16)
                    nc.vector.tensor_scalar(
                        oh[:],
                        iota[:],
                        lA[:, tcol : tcol + 1],
                        CA[:, tcol : tcol + 1],
                        OP.is_equal,
                        OP.mult,
                    )
                    ti = s * 4 + b
                    nc.tensor.matmul(
                        aggT[:],
                        m0e[:, bs],
                        oh[:],
                        start=(ti == 0),
                        stop=(ti == n_tiles_w - 1),
                    )

            # ---- window output: out_w = ssp(aggT.T @ Wout + bout) ----
            aggs = opool.tile([128, 128], dt.bfloat16)
            nc.vector.tensor_copy(aggs[:], aggT[:])
            op = pz.tile([128, NF], dt.float32, tag="z")
            if bout_nonzero:
                nc.tensor.matmul(op[:], ones[:], boutr[:], start=True, stop=False)
                nc.tensor.matmul(op[:], aggs[:], Wout[:], start=False, stop=True)
            else:
                nc.tensor.matmul(op[:], aggs[:], Wout[:], start=True, stop=True)
            eo = opool.tile([128, NF], dt.float32, tag="eo")
            nc.scalar.activation(eo[:], op[:], AF.Exp, bias=cmln2[:, 0:1])
            outs = opool.tile([128, NF], dt.float32, tag="fin")
            nc.scalar.activation(outs[:], eo[:], AF.Ln, bias=chalf[:, 0:1])
            nrows = min(WIN, NPC - w * WIN)
            nc.sync.dma_start(
                out_d[w * WIN : w * WIN + nrows, :], outs[:nrows, :]
            )

    if not nc.is_finalized():
        nc.finalize()
    return nc


def kernel(**inputs):
    from concourse.bass_utils import run_bass_kernel_spmd

    per_core, consts, gpad, E_pad, T_cols = _prep(inputs)
    bout_nonzero = bool(np.any(consts["boutr"]))

    no_gather = os.environ.get("CFCONV_NOGATHER", "0") == "1"
    nc = _build(gpad, E_pad, T_cols, bout_nonzero=bout_nonzero,
                no_gather=no_gather)

    in_maps = []
    for c in range(NCORES):
        m = dict(per_core[c])
        m.update(consts)
        in_maps.append(m)

    trace = os.environ.get("CFCONV_TRACE", "0") == "1"
    res = run_bass_kernel_spmd(nc, in_maps, list(range(NCORES)), trace=trace)
    if trace and res.exec_time_ns is not None:
        print(f"HW exec time: {res.exec_time_ns} ns")
        kernel.last_exec_time_ns = res.exec_time_ns
    kernel.last_results = res
    out = np.concatenate(
        [np.asarray(res.results[c]["out"]) for c in range(NCORES)], axis=0
    )
    return out.astype(np.float32)


# revision 18
# speedup vs baseline: 1.1211x; 1.1211x over previous
"""CFConv (SchNet continuous-filter conv) Trainium2 Bass kernel, 8-core SPMD.

v2 strategy (filter-major MLP + SBUF-resident h):
  - Host: bucket edges by dest node range (ind_i // 6250 -> core), within a
    core group by (128-node dest window, src-half); group sizes padded to
    128 (window totals to 512) so the edge stream is supertiles of 512.
  - Device: h = x @ Win computed in bf16 and kept RESIDENT in SBUF
    ([128, 392, 128]: partition = node%128, rank = node//128). Neighbor
    rows are fetched with SBUF-source dma_gather (transpose mode), which
    lands them filter-major [128 filt, E] and avoids the HBM-latency-bound
    per-row gather entirely.
  - Filter MLP runs filter-major with resident stationary weights:
    z1 = Wf1.T@fT, e1 = Exp(z1+b1), a1 = Ln(e1+1) [= softplus],
    z2 = Wf2.T@a1, e2 = Exp(z2 + b2 - ln2*sum(Wf2,0) - ln2),
    tt = Ln(e2 + 0.5) [= ssp(z2+b2) - ln2, both ln2 shifts folded].
    Exp and Ln share one ACT table (patched table sets -> no table thrash).
  - m0 = tt * hg (filter-major), PE-transposed per 128-edge block to
    edge-major, then scattered into aggT[filt, dest] PSUM via one-hot
    matmuls (one-hot = is_equal(iota, l)*C built on DVE, cutoff folded).
  - Window output: out = Ln(Exp(aggT.T @ Wout - ln2) + 0.5) -> DMA.
No cross-core collectives: each core owns 6250 output rows.
"""

import math
import os
import sys

import numpy as np

sys.path.insert(0, "/opt/trn_rl_repo")

N_ATOMS = 50000
N_EDGES = 1600000
DIM = 128
NF = 128
NG = 50
CUTOFF = 10.0
LOG2 = float(np.log(2.0))
NCORES = 8
NPC = N_ATOMS // NCORES  # 6250 nodes per core
WIN = 128  # dest-window size
NWIN = (NPC + WIN - 1) // WIN  # 49
JHALF = 25088  # gather index half boundary (196*128, int16-safe)
SUPER = 512  # edges per supertile
NPAD = 50176  # padded atom count (392*128)
NRANK = NPAD // 128  # 392
LNCHUNK = 2048  # ACT Ln batch size


def _prep(inputs):
    """Host-side bucketing/padding. Returns per-core arrays + constants."""
    import ml_dtypes

    bf16 = np.dtype(ml_dtypes.bfloat16)

    x = np.asarray(inputs["x"], dtype=np.float32)
    r = np.asarray(inputs["r_ij"], dtype=np.float32)
    f = np.asarray(inputs["f_ij"], dtype=np.float32)
    ii = np.asarray(inputs["ind_i"]).astype(np.int64)
    jj = np.asarray(inputs["ind_j"]).astype(np.int64)

    core = ii // NPC
    wloc = (ii - core * NPC) // WIN  # 0..48
    lloc = (ii - core * NPC - wloc * WIN).astype(np.float32)  # 0..127
    half = (jj >= JHALF).astype(np.int64)
    NG_GROUPS = NWIN * 2
    wg = wloc * 2 + half

    gkey = core * NG_GROUPS + wg
    order = np.lexsort((jj, gkey))  # sort by (core, window, half), then j
    counts = np.bincount(gkey, minlength=NCORES * NG_GROUPS).reshape(
        NCORES, NG_GROUPS
    )
    gmax = counts.max(axis=0)
    gpad = np.maximum(WIN, ((gmax + WIN - 1) // WIN) * WIN)  # [98], 128-mult
    # force each window's (lo+hi) total to a SUPER multiple
    for w in range(NWIN):
        tot = gpad[2 * w] + gpad[2 * w + 1]
        gpad[2 * w + 1] += (-tot) % SUPER
    offs = np.concatenate([[0], np.cumsum(gpad)])
    E_pad = int(offs[-1])
    T_cols = E_pad // 128

    sorted_gkey = gkey[order]
    first_idx = np.searchsorted(sorted_gkey, np.arange(NCORES * NG_GROUPS))
    rank = np.arange(N_EDGES) - first_idx[sorted_gkey]
    slot = offs[sorted_gkey % NG_GROUPS] + rank

    per_core = []
    for c in range(NCORES):
        sel = order[core[order] == c]
        sl = slot[core[order] == c]
        f_pad = np.zeros((E_pad, NG), dtype=np.float32)
        r_pad = np.full(E_pad, 15.0, dtype=np.float32)  # killed by r<10 mask
        l_pad = np.zeros(E_pad, dtype=np.float32)
        j_pad = np.zeros(E_pad, dtype=np.int16)
        f_pad[sl] = f[sel]
        r_pad[sl] = r[sel]
        l_pad[sl] = lloc[sel]
        j_pad[sl] = (jj[sel] - half[sel] * JHALF).astype(np.int16)
        per_core.append(
            dict(
                fT=np.ascontiguousarray(f_pad.T.astype(bf16)),  # [50, E_pad]
                rA=np.ascontiguousarray(r_pad.reshape(T_cols, 128).T),
                lA=np.ascontiguousarray(l_pad.reshape(T_cols, 128).T),
                jx=np.ascontiguousarray(
                    np.tile(j_pad.reshape(-1, 16).T, (8, 1))
                ),  # [128, E_pad//16]
            )
        )

    xT = np.zeros((DIM, NPAD), dtype=np.float32)
    xT[:, :N_ATOMS] = x.T
    Wf2 = np.asarray(inputs["Wf2"], dtype=np.float32)
    b2e = (
        np.asarray(inputs["bf2"], dtype=np.float32)
        - LOG2 * Wf2.sum(axis=0)
        - LOG2
    ).reshape(NF, 1)
    bout = np.asarray(inputs["bout"], dtype=np.float32)
    consts = dict(
        xT=np.ascontiguousarray(xT.astype(bf16)),
        Wf1=np.asarray(inputs["Wf1"], dtype=np.float32).astype(bf16),
        Wf2=Wf2.astype(bf16),
        Win=np.ascontiguousarray(
            np.asarray(inputs["Win"], dtype=np.float32).astype(bf16)
        ),
        Wout=np.ascontiguousarray(
            np.asarray(inputs["Wout"], dtype=np.float32).astype(bf16)
        ),
        b1=np.asarray(inputs["bf1"], dtype=np.float32).reshape(NF, 1),
        b2e=np.ascontiguousarray(b2e),
        boutr=np.ascontiguousarray(bout.reshape(1, NF)),
        ones=np.ones((1, 128), dtype=np.float32),
        iota=np.ascontiguousarray(
            np.broadcast_to(np.arange(128, dtype=np.float32), (128, 128))
            .astype(bf16)
            .copy()
        ),
        ident=np.eye(128, dtype=np.float32).astype(bf16),
    )
    return per_core, consts, gpad, E_pad, T_cols


def _patch_act_tables():
    """Strip Exp/Ln/Sin from every act table set except the combined
    natural_log_exp_and_others (Exp+Ln) and trig_and_small (Sin), so the
    table chooser never alternates tables between Exp and Ln ops. Set ids
    are positional, so keys/order are preserved."""
    import concourse.bacc as bacc_mod
    import concourse.mybir as mybir

    if getattr(bacc_mod, "_cfconv_act_patched", False):
        return
    AF = mybir.ActivationFunctionType
    orig = bacc_mod.get_activation_tables

    def patched(arch):
        tabs = orig(arch)
        strip = {AF.Exp, AF.Ln, AF.Sin}
        out = {}
        for name, fns in tabs.items():
            if name in ("natural_log_exp_and_others", "trig_and_small"):
                out[name] = fns
            else:
                out[name] = fns - strip
        return out

    bacc_mod.get_activation_tables = patched
    bacc_mod._cfconv_act_patched = True


def _build(gpad, E_pad, T_cols, bout_nonzero=False, no_gather=False):
    """Build the SPMD bass program (same for all cores)."""
    from contextlib import ExitStack

    import concourse.bacc as bacc
    import concourse.bass as bass
    import concourse.mybir as mybir
    import concourse.tile as tile

    _patch_act_tables()

    dt = mybir.dt
    AF = mybir.ActivationFunctionType
    OP = mybir.AluOpType

    nc = bacc.Bacc()

    # ---- I/O ----
    fT_d = nc.declare_dram_parameter("fT", [NG, E_pad], dt.bfloat16, isOutput=False)
    rA_d = nc.declare_dram_parameter("rA", [128, T_cols], dt.float32, isOutput=False)
    lA_d = nc.declare_dram_parameter("lA", [128, T_cols], dt.float32, isOutput=False)
    jx_d = nc.declare_dram_parameter(
        "jx", [128, E_pad // 16], dt.int16, isOutput=False
    )
    xT_d = nc.declare_dram_parameter("xT", [DIM, NPAD], dt.bfloat16, isOutput=False)
    Wf1_d = nc.declare_dram_parameter("Wf1", [NG, NF], dt.bfloat16, isOutput=False)
    Wf2_d = nc.declare_dram_parameter("Wf2", [NF, NF], dt.bfloat16, isOutput=False)
    Win_d = nc.declare_dram_parameter("Win", [DIM, NF], dt.bfloat16, isOutput=False)
    Wout_d = nc.declare_dram_parameter("Wout", [NF, NF], dt.bfloat16, isOutput=False)
    b1_d = nc.declare_dram_parameter("b1", [NF, 1], dt.float32, isOutput=False)
    b2e_d = nc.declare_dram_parameter("b2e", [NF, 1], dt.float32, isOutput=False)
    bout_d = nc.declare_dram_parameter("boutr", [1, NF], dt.float32, isOutput=False)
    ones_d = nc.declare_dram_parameter("ones", [1, 128], dt.float32, isOutput=False)
    iota_d = nc.declare_dram_parameter("iota", [128, 128], dt.bfloat16, isOutput=False)
    ident_d = nc.declare_dram_parameter(
        "ident", [128, 128], dt.bfloat16, isOutput=False
    )
    out_d = nc.declare_dram_parameter("out", [NPC, NF], dt.float32, isOutput=True)

    offs = np.concatenate([[0], np.cumsum(gpad)]).astype(int)
    JRANK = JHALF // 128  # 196

    with tile.TileContext(nc) as tc, ExitStack() as ctx:
        cpool = ctx.enter_context(tc.tile_pool(name="consts", bufs=1))
        meta = ctx.enter_context(tc.tile_pool(name="meta", bufs=1))
        scratch = ctx.enter_context(tc.tile_pool(name="scratch", bufs=1))
        scratch2 = ctx.enter_context(tc.tile_pool(name="scratch2", bufs=1))
        xm = ctx.enter_context(tc.tile_pool(name="xm", bufs=3))
        hpool = ctx.enter_context(tc.tile_pool(name="hsb", bufs=1))
        hgpool = ctx.enter_context(tc.tile_pool(name="hg", bufs=2))
        ftpool = ctx.enter_context(tc.tile_pool(name="ft", bufs=3))
        e1pool = ctx.enter_context(tc.tile_pool(name="e1w", bufs=1))
        e2pool = ctx.enter_context(tc.tile_pool(name="e2w", bufs=1))
        mepool = ctx.enter_context(tc.tile_pool(name="m0e", bufs=2))
        ohpool = ctx.enter_context(tc.tile_pool(name="oh", bufs=6))
        opool = ctx.enter_context(tc.tile_pool(name="outs", bufs=3))
        pz = ctx.enter_context(
            tc.tile_pool(name="pz", bufs=2, space=bass.MemorySpace.PSUM)
        )
        pz2 = ctx.enter_context(
            tc.tile_pool(name="pz2", bufs=2, space=bass.MemorySpace.PSUM)
        )
        pmt = ctx.enter_context(
            tc.tile_pool(name="pmt", bufs=2, space=bass.MemorySpace.PSUM)
        )
        pagg = ctx.enter_context(
            tc.tile_pool(name="pagg", bufs=2, space=bass.MemorySpace.PSUM)
        )

        # ---- constants ----
        Wf1 = cpool.tile([NG, NF], dt.bfloat16)
        nc.sync.dma_start(Wf1[:], Wf1_d[:])
        Wf2 = cpool.tile([NF, NF], dt.bfloat16)
        nc.sync.dma_start(Wf2[:], Wf2_d[:])
        Win = cpool.tile([DIM, NF], dt.bfloat16)
        nc.sync.dma_start(Win[:], Win_d[:])
        Wout = cpool.tile([NF, NF], dt.bfloat16)
        nc.sync.dma_start(Wout[:], Wout_d[:])
        b1 = cpool.tile([NF, 1], dt.float32)
        nc.sync.dma_start(b1[:], b1_d[:])
        b2e = cpool.tile([NF, 1], dt.float32)
        nc.sync.dma_start(b2e[:], b2e_d[:])
        boutr = cpool.tile([1, NF], dt.float32)
        nc.sync.dma_start(boutr[:], bout_d[:])
        ones = cpool.tile([1, 128], dt.float32)
        nc.sync.dma_start(ones[:], ones_d[:])
        iota = cpool.tile([128, 128], dt.bfloat16)
        nc.sync.dma_start(iota[:], iota_d[:])
        ident = cpool.tile([128, 128], dt.bfloat16)
        nc.sync.dma_start(ident[:], ident_d[:])
        chalf = cpool.tile([128, 1], dt.float32)
        nc.gpsimd.memset(chalf[:], 0.5)
        cmln2 = cpool.tile([128, 1], dt.float32)
        nc.gpsimd.memset(cmln2[:], -LOG2)

        # ---- per-edge metadata: l, C ----
        lA = meta.tile([128, T_cols], dt.float32)
        nc.sync.dma_start(lA[:], lA_d[:])
        rA = scratch.tile([128, T_cols], dt.float32)
        nc.sync.dma_start(rA[:], rA_d[:])
        jx = meta.tile([128, E_pad // 16], dt.int16)
        nc.sync.dma_start(jx[:], jx_d[:])

        CA = meta.tile([128, T_cols], dt.float32)
        # cos(pi*r/10) = sin(pi/2 - pi*r/10), arg in [-pi, pi] for r in
        # [0, 15]; C = (0.5*C0+0.5) * (r < 10).  Sin first -> trig table
        # loads once, everything after uses the Exp+Ln table.
        # In-place: rA <- pi/2 - pi*r/10; then r<10 <=> rA > -pi/2.
        nc.vector.tensor_scalar(
            rA[:], rA[:], float(-np.pi / CUTOFF), float(np.pi / 2), OP.mult, OP.add
        )
        nc.scalar.activation(CA[:], rA[:], AF.Sin)
        nc.vector.tensor_scalar(CA[:], CA[:], 0.5, 0.5, OP.mult, OP.add)
        msk = scratch2.tile([128, T_cols], dt.bfloat16)
        nc.vector.tensor_scalar(
            msk[:], rA[:], float(-np.pi / 2), None, OP.is_gt
        )
        nc.vector.tensor_tensor(CA[:], CA[:], msk[:], OP.mult)

        # ---- phase 1: h = x @ Win, bf16, resident in SBUF ----
        # h_lo/h_hi[p, r, :] = h[(rbase + r)*128 + p, :] (token layout,
        # tpr=128). Two tiles: a single source view must stay under 64 KiB
        # of free bytes per partition, and view offsets into the source are
        # not honored by the HW gather path.
        h_lo = hpool.tile([128, JRANK, NF], dt.bfloat16, tag="hlo")
        h_hi = hpool.tile([128, NRANK - JRANK, NF], dt.bfloat16, tag="hhi")
        for nb4 in range(NRANK // 4):  # 98 groups of 4 node-blocks
            xa = xm.tile([128, 4, 128], dt.bfloat16)
            nc.sync.dma_start(
                xa[:],
                xT_d[:, nb4 * 512 : (nb4 + 1) * 512].rearrange(
                    "p (b n) -> p b n", b=4
                ),
            )
            hp = pz.tile([128, 4, NF], dt.float32, tag="z")
            for b in range(4):
                nc.tensor.matmul(
                    hp[:, b, :], xa[:, b, :], Win[:], start=True, stop=True
                )
            r0 = nb4 * 4
            if r0 + 4 <= JRANK:
                nc.vector.tensor_copy(h_lo[:, r0 : r0 + 4, :], hp[:])
            else:
                nc.vector.tensor_copy(
                    h_hi[:, r0 - JRANK : r0 - JRANK + 4, :], hp[:]
                )

        tc.strict_bb_all_engine_barrier()

        # ---- phase 2: edge loop, one window (128 dest nodes) at a time ----
        for w in range(NWIN):
            g0 = int(gpad[2 * w])
            g1 = int(gpad[2 * w + 1])
            woff = int(offs[2 * w])
            wsize = g0 + g1
            ns = wsize // SUPER

            hg = hgpool.tile([128, 1, wsize], dt.bfloat16)
            if no_gather:
                nc.gpsimd.memset(hg[:], 1.0)
            else:
                # one SP=False gather per half; SWDGE descriptor
                # turnaround (~10 ns/idx) is the kernel's critical path.
                for src_t, hoff, hsz in ((h_lo, 0, g0), (h_hi, g0, g1)):
                    e0 = woff + hoff
                    nc.gpsimd.dma_gather(
                        hg[:, :, hoff : hoff + hsz],
                        src_t[:],
                        jx[:, e0 // 16 : (e0 + hsz) // 16],
                        hsz,
                        hsz,
                        NF,
                        transpose=True,
                        sbuf_tokens_per_rank=128,
                        sbuf_free_dim_per_rank=NF * 2,
                        single_packet=False,
                    )
            # stage 1: z1 = Wf1.T @ ft ; e1 = Exp(z1 + b1)
            e1w = e1pool.tile([128, wsize], dt.bfloat16)
            for s in range(ns):
                sl = slice(s * SUPER, (s + 1) * SUPER)
                ft = ftpool.tile([NG, SUPER], dt.bfloat16)
                nc.sync.dma_start(
                    ft[:], fT_d[:, woff + s * SUPER : woff + (s + 1) * SUPER]
                )
                z1 = pz.tile([128, SUPER], dt.float32, tag="z")
                nc.tensor.matmul(z1[:], Wf1[:], ft[:], start=True, stop=True)
                nc.scalar.activation(e1w[:, sl], z1[:], AF.Exp, bias=b1[:, 0:1])
            # stage 2: a1 = Ln(e1 + 1)  [in-place, batched]
            for c0 in range(0, wsize, LNCHUNK):
                c1 = min(c0 + LNCHUNK, wsize)
                nc.scalar.activation(
                    e1w[:, c0:c1], e1w[:, c0:c1], AF.Ln, bias=1.0
                )
            # stage 3: z2 = Wf2.T @ a1 ; e2 = Exp(z2 + b2e)
            e2w = e2pool.tile([128, wsize], dt.bfloat16)
            for s in range(ns):
                sl = slice(s * SUPER, (s + 1) * SUPER)
                z2 = pz2.tile([128, SUPER], dt.float32)
                nc.tensor.matmul(z2[:], Wf2[:], e1w[:, sl], start=True, stop=True)
                nc.scalar.activation(e2w[:, sl], z2[:], AF.Exp, bias=b2e[:, 0:1])
            # stage 4: tt = Ln(e2 + 0.5)  [= ssp(z2+b2) - ln2, in-place]
            for c0 in range(0, wsize, LNCHUNK):
                c1 = min(c0 + LNCHUNK, wsize)
                nc.scalar.activation(
                    e2w[:, c0:c1], e2w[:, c0:c1], AF.Ln, bias=chalf[:, 0:1]
                )

            # stage 5: m0 = tt * hg; transpose to edge-major; scatter
            aggT = pagg.tile([128, 128], dt.float32)
            n_tiles_w = wsize // 128
            for s in range(ns):
                sl = slice(s * SUPER, (s + 1) * SUPER)
                m0f = xm.tile([128, SUPER], dt.bfloat16)
                nc.vector.tensor_tensor(
                    m0f[:], e2w[:, sl], hg[:, 0, sl], OP.mult
                )
                m0T = pmt.tile([128, SUPER], dt.bfloat16)
                for b in range(4):
                    bs = slice(b * 128, (b + 1) * 128)
                    nc.tensor.transpose(m0T[:, bs], m0f[:, bs], ident[:])
                m0e = mepool.tile([128, SUPER], dt.bfloat16)
                nc.vector.tensor_copy(m0e[:], m0T[:])
                for b in range(4):
                    bs = slice(b * 128, (b + 1) * 128)
                    tcol = (woff + s * SUPER) // 128 + b
                    oh = ohpool.tile([128, 128], dt.bfloat16)
                    nc.vector.tensor_scalar(
                        oh[:],
                        iota[:],
                        lA[:, tcol : tcol + 1],
                        CA[:, tcol : tcol + 1],
                        OP.is_equal,
                        OP.mult,
                    )
                    ti = s * 4 + b
                    nc.tensor.matmul(
                        aggT[:],
                        m0e[:, bs],
                        oh[:],
                        start=(ti == 0),
                        stop=(ti == n_tiles_w - 1),
                    )

            # ---- window output: out_w = ssp(aggT.T @ Wout + bout) ----
            aggs = opool.tile([128, 128], dt.bfloat16)
            nc.vector.tensor_copy(aggs[:], aggT[:])
            op = pz.tile([128, NF], dt.float32, tag="z")
            if bout_nonzero:
                nc.tensor.matmul(op[:], ones[:], boutr[:], start=True, stop=False)
                nc.tensor.matmul(op[:], aggs[:], Wout[:], start=False, stop=True)
            else:
                nc.tensor.matmul(op[:], aggs[:], Wout[:], start=True, stop=True)
            eo = opool.tile([128, NF], dt.float32, tag="eo")
            nc.scalar.activation(eo[:], op[:], AF.Exp, bias=cmln2[:, 0:1])
            outs = opool.tile([128, NF], dt.float32, tag="fin")
            nc.scalar.activation(outs[:], eo[:], AF.Ln, bias=chalf[:, 0:1])
            nrows = min(WIN, NPC - w * WIN)
            nc.sync.dma_start(
                out_d[w * WIN : w * WIN + nrows, :], outs[:nrows, :]
            )

    if not nc.is_finalized():
        nc.finalize()
    return nc


def kernel(**inputs):
    from concourse.bass_utils import run_bass_kernel_spmd

    per_core, consts, gpad, E_pad, T_cols = _prep(inputs)
    bout_nonzero = bool(np.any(consts["boutr"]))

    no_gather = os.environ.get("CFCONV_NOGATHER", "0") == "1"
    nc = _build(gpad, E_pad, T_cols, bout_nonzero=bout_nonzero,
                no_gather=no_gather)

    in_maps = []
    for c in range(NCORES):
        m = dict(per_core[c])
        m.update(consts)
        in_maps.append(m)

    trace = os.environ.get("CFCONV_TRACE", "0") == "1"
    res = run_bass_kernel_spmd(nc, in_maps, list(range(NCORES)), trace=trace)
    if trace and res.exec_time_ns is not None:
        print(f"HW exec time: {res.exec_time_ns} ns")
        kernel.last_exec_time_ns = res.exec_time_ns
    kernel.last_results = res
    out = np.concatenate(
        [np.asarray(res.results[c]["out"]) for c in range(NCORES)], axis=0
    )
    return out.astype(np.float32)
